# revision 7
# baseline (speedup 1.0000x reference)
"""2-layer GAT + per-graph max-pool + MLP head on 8 Trainium2 cores.

Sharding: core k owns nodes [1250k, 1250k+1250) = graphs [8k, 8k+8)
(graph_ids are contiguous, 8 graphs = exactly 1250 nodes). Edges are
partitioned by dst node and sorted by dst on the host; all params and
the node-feature table are replicated so src gathers stay core-local.
Segment softmax / aggregation are computed with one-hot matmuls on the
PE array; an AllGather shares the inter-layer activations.
"""

import json

import numpy as np

import concourse.bass as bass
import concourse.mybir as mybir
import concourse.tile as tile
from concourse.bass_utils import run_bass_kernel_spmd
from concourse.masks import make_identity

# ---------------------------------------------------------------- problem dims
N, E, F, D, H, B, CLS = 10000, 160000, 128, 128, 4, 64, 2
FF = 768
SLOPE = 0.2
NCORES = 8
NLOC = N // NCORES            # 1250 nodes per core
NT = (NLOC + 127) // 128      # 10 dst tiles per core
TILE_CNT = [128] * 9 + [NLOC - 9 * 128]   # rows per dst tile
ROW = D * H + H               # 516: [h | el]
# local graph boundaries (same for every core): ceil(j * 10000/64) - 1250*k
GB = [0, 157, 313, 469, 625, 782, 938, 1094, 1250]
PAD_DST = 999                 # dst_local for padding slots (no one-hot match)

f32 = mybir.dt.float32
i32 = mybir.dt.int32
AX = mybir.AxisListType.X
OP = mybir.AluOpType


# ------------------------------------------------------------ BIR legalization
# This walrus build caps semaphore waits at 1 per instruction (2 for
# EventSemaphore); Tile's kernel-tail drain can carry more. Split the
# excess onto preceding single-wait NoOps on the same engine.
def _legalize_bir(bir: dict) -> None:
    caps = {"EventSemaphore": 2}
    for f in bir.get("functions", []):
        for blk in f.get("blocks", []):
            out = []
            ctr = 0
            for ins in blk.get("instructions", []):
                si = ins.get("sync_info")
                if si:
                    waits = si.get("on_wait") or []
                    cap = caps.get(ins.get("opcode"), 1)
                    if len(waits) > cap:
                        eq = [i for i, w in enumerate(waits)
                              if "eq" in (w.get("wait_mode") or "")]
                        ge = [i for i in range(len(waits)) if i not in eq]
                        assert len(eq) <= cap
                        keep = (eq + ge)[:cap]
                        for i in [i for i in ge if i not in keep]:
                            ctr += 1
                            out.append({
                                "debug": ins.get("debug"),
                                "engine": ins["engine"],
                                "ins": [], "outs": [],
                                "name": f"{ins['name']}_ws{ctr}",
                                "opcode": "NoOp",
                                "sync_info": {"on_wait": [waits[i]],
                                              "on_update": []},
                            })
                        si["on_wait"] = [waits[i] for i in sorted(keep)]
                out.append(ins)
            blk["instructions"] = out


_orig_to_json_bytes = bass.Bass.to_json_bytes


def _to_json_bytes_patched(self):
    bir = json.loads(_orig_to_json_bytes(self))
    _legalize_bir(bir)
    return json.dumps(bir).encode()


if bass.Bass.to_json_bytes is not _to_json_bytes_patched:
    bass.Bass.to_json_bytes = _to_json_bytes_patched


# ---------------------------------------------------------------- device build
def build_nc(C: int, debug: bool = False) -> bass.Bass:
    """Build the SPMD kernel; C = chunks (of 128 edge slots) per dst tile."""
    nc = bass.Bass()

    xT = nc.dram_tensor("xT", [F, N], f32, kind="ExternalInput")
    xT_own = nc.dram_tensor("xT_own", [F, NLOC], f32, kind="ExternalInput")
    srcidx = nc.dram_tensor("srcidx", [NT * 128, C], i32, kind="ExternalInput")
    dstloc = nc.dram_tensor("dstloc", [NT * 128, C], i32, kind="ExternalInput")
    w1aug = nc.dram_tensor("w1aug", [F, ROW], f32, kind="ExternalInput")
    war1 = nc.dram_tensor("war1", [F, H], f32, kind="ExternalInput")
    w2aug = nc.dram_tensor("w2aug", [D, ROW], f32, kind="ExternalInput")
    war2 = nc.dram_tensor("war2", [D, H], f32, kind="ExternalInput")
    b1m_d = nc.dram_tensor("b1m", [1, D], f32, kind="ExternalInput")
    b2_d = nc.dram_tensor("b2", [1, H * D], f32, kind="ExternalInput")
    lng_d = nc.dram_tensor("lng", [1, D], f32, kind="ExternalInput")
    lnb_d = nc.dram_tensor("lnb", [1, D], f32, kind="ExternalInput")
    wa_d = nc.dram_tensor("wa", [H * D, D], f32, kind="ExternalInput")
    ba_d = nc.dram_tensor("ba", [1, D], f32, kind="ExternalInput")
    wb_d = nc.dram_tensor("wb", [D, FF], f32, kind="ExternalInput")
    bb_d = nc.dram_tensor("bb", [1, FF], f32, kind="ExternalInput")
    l2g_d = nc.dram_tensor("l2g", [1, FF], f32, kind="ExternalInput")
    l2b_d = nc.dram_tensor("l2b", [1, FF], f32, kind="ExternalInput")
    wc_d = nc.dram_tensor("wc", [FF, CLS], f32, kind="ExternalInput")
    bc_d = nc.dram_tensor("bc", [1, CLS], f32, kind="ExternalInput")

    aw_out = nc.dram_tensor("aw_out", [NT * 128, C * H], f32, kind="ExternalOutput")
    if debug:
        dbg_haug1 = nc.dram_tensor("dbg_haug1", [N, ROW], f32, kind="ExternalOutput")
        dbg_x2 = nc.dram_tensor("dbg_x2", [NLOC, D], f32, kind="ExternalOutput")
        dbg_x2full = nc.dram_tensor("dbg_x2full", [N, D], f32, kind="ExternalOutput")
        dbg_rden1 = nc.dram_tensor("dbg_rden1", [NT * 128, H], f32, kind="ExternalOutput")
        dbg_h2 = nc.dram_tensor("dbg_h2", [NT * 128, 512], f32, kind="ExternalOutput")
        dbg_er2 = nc.dram_tensor("dbg_er2", [NT * 128, H], f32, kind="ExternalOutput")
        dbg_agg = nc.dram_tensor("dbg_agg", [NT * 128, 512], f32, kind="ExternalOutput")
    probs_out = nc.dram_tensor("probs", [B // NCORES, CLS], f32, kind="ExternalOutput")

    def bcast(dram_t, parts, k):
        # [1, k] DRAM row -> [parts, k] via partition-stride-0 DMA
        return bass.AP(tensor=dram_t[:].tensor, offset=0, ap=[[0, parts], [1, k]])

    with tile.TileContext(nc) as tc:
        with (
            tc.tile_pool(name="consts", bufs=1) as consts,
            tc.tile_pool(name="dram", bufs=1, space="DRAM") as dram,
            tc.tile_pool(name="persist", bufs=1) as persist,
        ):
            # ---------------- constants
            identity = consts.tile([128, 128], f32)
            make_identity(nc, identity[:])
            iota_row_i = consts.tile([128, 128], i32)
            nc.gpsimd.iota(iota_row_i[:], pattern=[[1, 128]], channel_multiplier=0)
            iota_row = consts.tile([128, 128], f32)
            nc.vector.tensor_copy(iota_row[:], iota_row_i[:])
            iota_col_i = consts.tile([128, 1], i32)
            nc.gpsimd.iota(iota_col_i[:], pattern=[[0, 1]], channel_multiplier=1)
            iota_col = consts.tile([128, 1], f32)
            nc.vector.tensor_copy(iota_col[:], iota_col_i[:])
            ones_col = consts.tile([128, 1], f32)
            nc.vector.memset(ones_col[:], 1.0)

            w1aug_sb = consts.tile([F, ROW], f32)
            nc.sync.dma_start(out=w1aug_sb[:], in_=w1aug[:])
            war1_sb = consts.tile([F, H], f32)
            nc.sync.dma_start(out=war1_sb[:], in_=war1[:])
            w2aug_sb = consts.tile([D, ROW], f32)
            nc.sync.dma_start(out=w2aug_sb[:], in_=w2aug[:])
            war2_sb = consts.tile([D, H], f32)
            nc.sync.dma_start(out=war2_sb[:], in_=war2[:])
            b1m_sb = consts.tile([128, D], f32)
            nc.sync.dma_start(out=b1m_sb[:], in_=bcast(b1m_d, 128, D))
            b2_sb = consts.tile([128, H * D], f32)
            nc.sync.dma_start(out=b2_sb[:], in_=bcast(b2_d, 128, H * D))
            lng_sb = consts.tile([128, D], f32)
            nc.sync.dma_start(out=lng_sb[:], in_=bcast(lng_d, 128, D))
            lnb_sb = consts.tile([128, D], f32)
            nc.sync.dma_start(out=lnb_sb[:], in_=bcast(lnb_d, 128, D))
            wa_sb = consts.tile([128, H, D], f32)
            nc.sync.dma_start(out=wa_sb[:], in_=wa_d.rearrange("(b k) n -> k b n", k=128))
            ba_sb = consts.tile([8, D], f32)
            nc.sync.dma_start(out=ba_sb[:], in_=bcast(ba_d, 8, D))
            wb_sb = consts.tile([D, FF], f32)
            nc.sync.dma_start(out=wb_sb[:], in_=wb_d[:])
            bb_sb = consts.tile([8, FF], f32)
            nc.sync.dma_start(out=bb_sb[:], in_=bcast(bb_d, 8, FF))
            l2g_sb = consts.tile([8, FF], f32)
            nc.sync.dma_start(out=l2g_sb[:], in_=bcast(l2g_d, 8, FF))
            l2b_sb = consts.tile([8, FF], f32)
            nc.sync.dma_start(out=l2b_sb[:], in_=bcast(l2b_d, 8, FF))
            wc_sb = consts.tile([128, 6, CLS], f32)
            nc.sync.dma_start(out=wc_sb[:], in_=wc_d.rearrange("(b k) n -> k b n", k=128))
            bc_sb = consts.tile([8, CLS], f32)
            nc.sync.dma_start(out=bc_sb[:], in_=bcast(bc_d, 8, CLS))
            xT_own_sb = consts.tile([F, NLOC], f32)
            nc.sync.dma_start(out=xT_own_sb[:], in_=xT_own[:])

            # persistent state
            er2_all = persist.tile([128, NT, H], f32)
            pooledT = persist.tile([128, H, 8], f32)
            nc.vector.memset(pooledT[:], -3.0e38)

            # internal DRAM
            haug1 = dram.tile([N, ROW], f32)
            haug2 = dram.tile([N, ROW], f32)
            x2_shard = dram.tile([NLOC, D], f32)
            x2_full = dram.tile([N, D], f32, addr_space="Shared")

            # ============================ P0: haug1 = x @ [W1|Wal1], full N
            with (
                tc.tile_pool(name="p0", bufs=2) as p0,
                tc.tile_pool(name="p0ps", bufs=2, space="PSUM") as p0ps,
                tc.tile_pool(name="xtp", bufs=1) as xtp,
            ):
                xT_sb = xtp.tile([F, N], f32)
                nc.sync.dma_start(out=xT_sb[:], in_=xT[:])
                for i in range((N + 127) // 128):
                    cnt = min(128, N - i * 128)
                    ph = p0ps.tile([128, 512], f32, tag="ph")
                    pe = p0ps.tile([128, H], f32, tag="pe")
                    lhs = xT_sb[:, i * 128:i * 128 + cnt]
                    nc.tensor.matmul(ph[:cnt, :], lhsT=lhs, rhs=w1aug_sb[:, 0:512],
                                     start=True, stop=True)
                    nc.tensor.matmul(pe[:cnt, :], lhsT=lhs, rhs=w1aug_sb[:, 512:ROW],
                                     start=True, stop=True)
                    hrow = p0.tile([128, ROW], f32, tag="hrow")
                    nc.scalar.copy(out=hrow[:cnt, 0:512], in_=ph[:cnt, :])
                    nc.vector.tensor_copy(out=hrow[:cnt, 512:ROW], in_=pe[:cnt, :])
                    nc.sync.dma_start(out=haug1[i * 128:i * 128 + cnt, :],
                                      in_=hrow[:cnt, :])

            # ============================ edge-loop helper (shared by layers)
            def edge_layer(haug, war_sb, layer):
                """Per dst tile: gather src rows, attention, aggregate.

                layer=1: returns per-tile x2 (LN'd head-mean) written to
                x2_shard + er2_all. layer=2: writes aw, updates pooledT.
                """
                with (
                    tc.tile_pool(name=f"hs{layer}", bufs=2) as hsp,
                    tc.tile_pool(name=f"oh{layer}", bufs=2) as ohp,
                    tc.tile_pool(name=f"wk{layer}", bufs=3) as wk,
                    tc.tile_pool(name=f"soh{layer}", bufs=4) as sohp,
                    tc.tile_pool(name=f"sm{layer}", bufs=8) as sm,
                    tc.tile_pool(name=f"ps{layer}", bufs=2, space="PSUM") as ps,
                    tc.tile_pool(name=f"pt{layer}", bufs=2, space="PSUM") as pst,
                ):
                    for t in range(NT):
                        cnt = TILE_CNT[t]
                        r0 = t * 128
                        # --- load index data
                        idx_sb = wk.tile([128, C], i32, tag="idx")
                        nc.sync.dma_start(out=idx_sb[:], in_=srcidx[r0:r0 + 128, :])
                        dl_sb = wk.tile([128, C], i32, tag="dl")
                        nc.sync.dma_start(out=dl_sb[:], in_=dstloc[r0:r0 + 128, :])
                        dlf = wk.tile([128, C], f32, tag="dlf")
                        nc.vector.tensor_copy(dlf[:], dl_sb[:])
                        # --- er for this tile's dst rows
                        er_sb = sm.tile([128, H], f32, tag="er")
                        nc.vector.memset(er_sb[:], 0.0)
                        per = pst.tile([128, H], f32, tag="per", bufs=1)
                        if layer == 1:
                            nc.tensor.matmul(per[:cnt, :],
                                             lhsT=xT_own_sb[:, r0:r0 + cnt],
                                             rhs=war_sb[:], start=True, stop=True)
                            nc.vector.tensor_copy(er_sb[:cnt, :], per[:cnt, :])
                        else:
                            nc.vector.tensor_copy(er_sb[:cnt, :], er2_all[:cnt, t, :])
                        # --- gather src rows (h | el)
                        hsrc = hsp.tile([128, C, ROW], f32, tag="hsrc")
                        for j in range(C):
                            nc.gpsimd.indirect_dma_start(
                                out=hsrc[:, j, :], out_offset=None, in_=haug[:],
                                in_offset=bass.IndirectOffsetOnAxis(
                                    ap=idx_sb[:, j:j + 1], axis=0),
                            )
                        ohT = ohp.tile([128, C * 128], f32, tag="ohT")
                        extile = wk.tile([128, C * H], f32, tag="ex")
                        pagg = ps.tile([128, 512], f32, tag="pagg", bufs=1)
                        pden = ps.tile([128, H], f32, tag="pden", bufs=1)
                        for j in range(C):
                            # transposed dst_local -> one-hot^T [d, e]
                            pdT = pst.tile([128, 128], f32, tag="pdT")
                            nc.tensor.transpose(
                                out=pdT[:], in_=dlf[:, j:j + 1].to_broadcast([128, 128]),
                                identity=identity[:])
                            ohTj = ohT[:, j * 128:(j + 1) * 128]
                            nc.vector.tensor_tensor(
                                out=ohTj, in0=pdT[:],
                                in1=iota_col[:, 0:1].to_broadcast([128, 128]),
                                op=OP.is_equal)
                            # er broadcast to edges: ohT^T @ er
                            perb = pst.tile([128, H], f32, tag="perb")
                            nc.tensor.matmul(perb[:], lhsT=ohTj, rhs=er_sb[:],
                                             start=True, stop=True)
                            # e = leaky(el_src + er_dst); ex = exp(e)
                            ej = sm.tile([128, H], f32, tag="ej")
                            nc.vector.tensor_tensor(out=ej[:], in0=hsrc[:, j, 512:ROW],
                                                    in1=perb[:], op=OP.add)
                            nc.vector.scalar_tensor_tensor(
                                out=ej[:], in0=ej[:], scalar=SLOPE, in1=ej[:],
                                op0=OP.mult, op1=OP.max)
                            exj = extile[:, j * H:(j + 1) * H]
                            nc.scalar.activation(out=exj, in_=ej[:],
                                                 func=mybir.ActivationFunctionType.Exp)
                            # one-hot + scaled rhs aggregation matmuls
                            oh = sohp.tile([128, 128], f32, tag="oh")
                            nc.vector.tensor_scalar(
                                out=oh[:], in0=iota_row[:],
                                scalar1=dlf[:, j:j + 1], scalar2=None,
                                op0=OP.is_equal)
                            nc.tensor.matmul(
                                pden[:], lhsT=oh[:], rhs=exj,
                                start=(j == 0), stop=(j == C - 1),
                                skip_group_check=True)
                            sh = sohp.tile([128, 512], f32, tag="sh")
                            for h in range(H):
                                if h % 2 == 0:
                                    nc.scalar.activation(
                                        out=sh[:, h * 128:(h + 1) * 128],
                                        in_=hsrc[:, j, h * 128:(h + 1) * 128],
                                        func=mybir.ActivationFunctionType.Copy,
                                        scale=extile[:, j * H + h:j * H + h + 1])
                                else:
                                    nc.vector.tensor_scalar(
                                        out=sh[:, h * 128:(h + 1) * 128],
                                        in0=hsrc[:, j, h * 128:(h + 1) * 128],
                                        scalar1=extile[:, j * H + h:j * H + h + 1],
                                        scalar2=None, op0=OP.mult)
                            nc.tensor.matmul(
                                pagg[:], lhsT=oh[:], rhs=sh[:],
                                start=(j == 0), stop=(j == C - 1),
                                skip_group_check=True)
                        # --- denominators
                        rden = sm.tile([128, H], f32, tag="rden")
                        nc.vector.tensor_scalar(out=rden[:], in0=pden[:], scalar1=1e-9,
                                                scalar2=None, op0=OP.max)
                        nc.vector.reciprocal(out=rden[:], in_=rden[:])
                        if debug and layer == 1:
                            nc.sync.dma_start(out=dbg_rden1[r0:r0 + 128, :], in_=rden[:])
                            aggd = wk.tile([128, 512], f32, tag="aggd")
                            nc.vector.tensor_copy(aggd[:], pagg[:])
                            nc.sync.dma_start(out=dbg_agg[r0:r0 + 128, :], in_=aggd[:])

                        if layer == 1:
                            # hm = 0.25 * sum_h agg_h*rden_h + b1m
                            m0 = wk.tile([128, D], f32, tag="m0")
                            m1 = wk.tile([128, D], f32, tag="m1")
                            nc.scalar.activation(out=m0[:], in_=pagg[:, 0:128],
                                                 func=mybir.ActivationFunctionType.Copy,
                                                 scale=rden[:, 0:1])
                            nc.scalar.activation(out=m1[:], in_=pagg[:, 128:256],
                                                 func=mybir.ActivationFunctionType.Copy,
                                                 scale=rden[:, 1:2])
                            nc.vector.tensor_add(m0[:], m0[:], m1[:])
                            nc.scalar.activation(out=m1[:], in_=pagg[:, 256:384],
                                                 func=mybir.ActivationFunctionType.Copy,
                                                 scale=rden[:, 2:3])
                            nc.vector.tensor_add(m0[:], m0[:], m1[:])
                            nc.scalar.activation(out=m1[:], in_=pagg[:, 384:512],
                                                 func=mybir.ActivationFunctionType.Copy,
                                                 scale=rden[:, 3:4])
                            nc.vector.tensor_add(m0[:], m0[:], m1[:])
                            hm = wk.tile([128, D], f32, tag="hm")
                            nc.vector.scalar_tensor_tensor(
                                out=hm[:], in0=m0[:], scalar=0.25, in1=b1m_sb[:],
                                op0=OP.mult, op1=OP.add)
                            # LayerNorm over D
                            mu = sm.tile([128, 1], f32, tag="mu")
                            nc.vector.tensor_reduce(out=mu[:], in_=hm[:], axis=AX,
                                                    op=OP.add)
                            nc.scalar.mul(out=mu[:], in_=mu[:], mul=1.0 / D)
                            cen = wk.tile([128, D], f32, tag="cen")
                            nc.vector.tensor_scalar(out=cen[:], in0=hm[:],
                                                    scalar1=mu[:, 0:1], scalar2=None,
                                                    op0=OP.subtract)
                            sq = wk.tile([128, D], f32, tag="sq")
                            nc.vector.tensor_tensor(out=sq[:], in0=cen[:], in1=cen[:],
                                                    op=OP.mult)
                            var = sm.tile([128, 1], f32, tag="var")
                            nc.vector.tensor_reduce(out=var[:], in_=sq[:], axis=AX,
                                                    op=OP.add)
                            std = sm.tile([128, 1], f32, tag="std")
                            nc.vector.tensor_scalar(out=var[:], in0=var[:],
                                                    scalar1=1.0 / D, scalar2=1e-5,
                                                    op0=OP.mult, op1=OP.add)
                            nc.scalar.activation(out=std[:], in_=var[:],
                                                 func=mybir.ActivationFunctionType.Sqrt)
                            nc.vector.reciprocal(out=std[:], in_=std[:])
                            x2t = wk.tile([128, D], f32, tag="x2t")
                            nc.vector.tensor_scalar(out=x2t[:], in0=cen[:],
                                                    scalar1=std[:, 0:1], scalar2=None,
                                                    op0=OP.mult)
                            nc.vector.tensor_tensor(out=x2t[:], in0=x2t[:],
                                                    in1=lng_sb[:], op=OP.mult)
                            nc.vector.tensor_tensor(out=x2t[:], in0=x2t[:],
                                                    in1=lnb_sb[:], op=OP.add)
                            nc.vector.scalar_tensor_tensor(
                                out=x2t[:], in0=x2t[:], scalar=SLOPE, in1=x2t[:],
                                op0=OP.mult, op1=OP.max)
                            nc.sync.dma_start(out=x2_shard[r0:r0 + cnt, :],
                                              in_=x2t[:cnt, :])
                            if debug:
                                nc.sync.dma_start(out=dbg_x2[r0:r0 + cnt, :],
                                                  in_=x2t[:cnt, :])
                            # er2 for this tile = x2 @ War2 (needs x2^T)
                            pxT = pst.tile([128, 128], f32, tag="pdT")
                            nc.tensor.transpose(out=pxT[:, :cnt], in_=x2t[:cnt, :],
                                                identity=identity[:cnt, :cnt])
                            x2T = wk.tile([128, 128], f32, tag="x2T")
                            nc.vector.memset(x2T[:], 0.0)
                            nc.vector.tensor_copy(x2T[:, :cnt], pxT[:, :cnt])
                            per2 = pst.tile([128, H], f32, tag="per", bufs=1)
                            nc.tensor.matmul(per2[:cnt, :], lhsT=x2T[:, :cnt],
                                             rhs=war2_sb[:], start=True, stop=True)
                            nc.vector.memset(er2_all[:, t, :], 0.0)
                            nc.vector.tensor_copy(er2_all[:cnt, t, :], per2[:cnt, :])
                            if debug:
                                nc.sync.dma_start(out=dbg_er2[r0:r0 + 128, :],
                                                  in_=er2_all[:, t, :])
                        else:
                            # h2 = leaky(agg*rden + b2)
                            h2 = wk.tile([128, 512], f32, tag="h2")
                            for h in range(H):
                                blk = h2[:, h * 128:(h + 1) * 128]
                                nc.scalar.activation(
                                    out=blk, in_=pagg[:, h * 128:(h + 1) * 128],
                                    func=mybir.ActivationFunctionType.Copy,
                                    scale=rden[:, h:h + 1])
                            nc.vector.tensor_tensor(out=h2[:], in0=h2[:], in1=b2_sb[:],
                                                    op=OP.add)
                            nc.vector.scalar_tensor_tensor(
                                out=h2[:], in0=h2[:], scalar=SLOPE, in1=h2[:],
                                op0=OP.mult, op1=OP.max)
                            if debug:
                                nc.sync.dma_start(out=dbg_h2[r0:r0 + 128, :], in_=h2[:])
                            # attention weights out: a = ex * rden[dst]
                            aw_sb = wk.tile([128, C * H], f32, tag="aw")
                            for j in range(C):
                                pa = pst.tile([128, H], f32, tag="perb")
                                nc.tensor.matmul(pa[:],
                                                 lhsT=ohT[:, j * 128:(j + 1) * 128],
                                                 rhs=rden[:], start=True, stop=True)
                                nc.vector.tensor_tensor(
                                    out=aw_sb[:, j * H:(j + 1) * H],
                                    in0=extile[:, j * H:(j + 1) * H], in1=pa[:],
                                    op=OP.mult)
                            nc.sync.dma_start(out=aw_out[r0:r0 + 128, :], in_=aw_sb[:])
                            # per-graph max pool (feature-major via PE transpose)
                            for h in range(H):
                                pT = pst.tile([128, 128], f32, tag="pdT")
                                nc.tensor.transpose(
                                    out=pT[:, :cnt], in_=h2[:cnt, h * 128:(h + 1) * 128],
                                    identity=identity[:cnt, :cnt])
                                for g in range(8):
                                    lo = max(GB[g], r0) - r0
                                    hi = min(GB[g + 1], r0 + cnt) - r0
                                    if lo >= hi:
                                        continue
                                    red = sm.tile([128, 1], f32, tag="red")
                                    nc.vector.tensor_reduce(
                                        out=red[:], in_=pT[:, lo:hi], axis=AX, op=OP.max)
                                    nc.vector.tensor_tensor(
                                        out=pooledT[:, h, g:g + 1],
                                        in0=pooledT[:, h, g:g + 1], in1=red[:],
                                        op=OP.max)

            # ============================ P1: layer-1 edge loop
            edge_layer(haug1, war1_sb, layer=1)

            # ============================ AllGather x2 shards
            nc.gpsimd.collective_compute(
                "AllGather", OP.bypass,
                replica_groups=[list(range(NCORES))],
                ins=[x2_shard.opt()], outs=[x2_full.opt()],
            )

            if debug:
                with tc.tile_pool(name="dbgp", bufs=2) as dbgp:
                    for i in range((N + 127) // 128):
                        cnt = min(128, N - i * 128)
                        tmp = dbgp.tile([128, ROW], f32, tag="tmp")
                        nc.sync.dma_start(out=tmp[:cnt, :], in_=haug1[i * 128:i * 128 + cnt, :])
                        nc.sync.dma_start(out=dbg_haug1[i * 128:i * 128 + cnt, :], in_=tmp[:cnt, :])
                        tmp2 = dbgp.tile([128, D], f32, tag="tmp2")
                        nc.sync.dma_start(out=tmp2[:cnt, :], in_=x2_full[i * 128:i * 128 + cnt, :])
                        nc.sync.dma_start(out=dbg_x2full[i * 128:i * 128 + cnt, :], in_=tmp2[:cnt, :])

            # ============================ P2: haug2 = x2 @ [W2|Wal2], full N
            with (
                tc.tile_pool(name="p2", bufs=3) as p2,
                tc.tile_pool(name="p2ps", bufs=2, space="PSUM") as p2ps,
            ):
                for s in range(NCORES):
                    for t in range(NT):
                        cnt = TILE_CNT[t]
                        g0 = s * NLOC + t * 128
                        x2t_sb = p2.tile([128, D], f32, tag="x2t")
                        nc.sync.dma_start(out=x2t_sb[:cnt, :],
                                          in_=x2_full[g0:g0 + cnt, :])
                        pT = p2ps.tile([128, 128], f32, tag="pT")
                        nc.tensor.transpose(out=pT[:, :cnt], in_=x2t_sb[:cnt, :],
                                            identity=identity[:cnt, :cnt])
                        x2T_sb = p2.tile([128, 128], f32, tag="x2T")
                        nc.scalar.copy(out=x2T_sb[:, :cnt], in_=pT[:, :cnt])
                        ph = p2ps.tile([128, 512], f32, tag="ph")
                        pe = p2ps.tile([128, H], f32, tag="pe")
                        nc.tensor.matmul(ph[:cnt, :], lhsT=x2T_sb[:, :cnt],
                                         rhs=w2aug_sb[:, 0:512], start=True, stop=True)
                        nc.tensor.matmul(pe[:cnt, :], lhsT=x2T_sb[:, :cnt],
                                         rhs=w2aug_sb[:, 512:ROW], start=True, stop=True)
                        hrow = p2.tile([128, ROW], f32, tag="hrow")
                        nc.scalar.copy(out=hrow[:cnt, 0:512], in_=ph[:cnt, :])
                        nc.vector.tensor_copy(out=hrow[:cnt, 512:ROW], in_=pe[:cnt, :])
                        nc.sync.dma_start(out=haug2[g0:g0 + cnt, :], in_=hrow[:cnt, :])

            # ============================ P3: layer-2 edge loop
            edge_layer(haug2, war2_sb, layer=2)

            # ============================ P4: head MLP on pooled [8, 512]
            with (
                tc.tile_pool(name="p4", bufs=1) as p4,
                tc.tile_pool(name="p4ps", bufs=1, space="PSUM") as p4ps,
            ):
                pz1 = p4ps.tile([8, D], f32, tag="pz1")
                for h in range(H):
                    nc.tensor.matmul(pz1[:], lhsT=pooledT[:, h, :], rhs=wa_sb[:, h, :],
                                     start=(h == 0), stop=(h == H - 1))
                z1 = p4.tile([8, D], f32)
                nc.vector.tensor_tensor(out=z1[:], in0=pz1[:], in1=ba_sb[:], op=OP.add)
                pz1T = p4ps.tile([128, 8], f32, tag="pz1T")
                nc.tensor.transpose(out=pz1T[:], in_=z1[:], identity=identity[:8, :8])
                z1T = p4.tile([128, 8], f32)
                nc.vector.tensor_copy(z1T[:], pz1T[:])
                pz2a = p4ps.tile([8, 384], f32, tag="pz2a")
                pz2b = p4ps.tile([8, 384], f32, tag="pz2b")
                nc.tensor.matmul(pz2a[:], lhsT=z1T[:], rhs=wb_sb[:, 0:384],
                                 start=True, stop=True)
                nc.tensor.matmul(pz2b[:], lhsT=z1T[:], rhs=wb_sb[:, 384:FF],
                                 start=True, stop=True)
                z2 = p4.tile([8, FF], f32)
                nc.vector.tensor_copy(z2[:, 0:384], pz2a[:])
                nc.vector.tensor_copy(z2[:, 384:FF], pz2b[:])
                nc.vector.tensor_tensor(out=z2[:], in0=z2[:], in1=bb_sb[:], op=OP.add)
                # LayerNorm over FF
                mu = p4.tile([8, 1], f32)
                nc.vector.tensor_reduce(out=mu[:], in_=z2[:], axis=AX, op=OP.add)
                nc.scalar.mul(out=mu[:], in_=mu[:], mul=1.0 / FF)
                nc.vector.tensor_scalar(out=z2[:], in0=z2[:], scalar1=mu[:, 0:1],
                                        scalar2=None, op0=OP.subtract)
                sq = p4.tile([8, FF], f32)
                nc.vector.tensor_tensor(out=sq[:], in0=z2[:], in1=z2[:], op=OP.mult)
                var = p4.tile([8, 1], f32)
                nc.vector.tensor_reduce(out=var[:], in_=sq[:], axis=AX, op=OP.add)
                std = p4.tile([8, 1], f32)
                nc.vector.tensor_scalar(out=var[:], in0=var[:],
                                        scalar1=1.0 / FF, scalar2=1e-5,
                                        op0=OP.mult, op1=OP.add)
                nc.scalar.activation(out=std[:], in_=var[:],
                                     func=mybir.ActivationFunctionType.Sqrt)
                nc.vector.reciprocal(out=std[:], in_=std[:])
                nc.vector.tensor_scalar(out=z2[:], in0=z2[:], scalar1=std[:, 0:1],
                                        scalar2=None, op0=OP.mult)
                nc.vector.tensor_tensor(out=z2[:], in0=z2[:], in1=l2g_sb[:], op=OP.mult)
                nc.vector.tensor_tensor(out=z2[:], in0=z2[:], in1=l2b_sb[:], op=OP.add)
                nc.vector.scalar_tensor_tensor(out=z2[:], in0=z2[:], scalar=SLOPE,
                                               in1=z2[:], op0=OP.mult, op1=OP.max)
                # z3 = z2 @ Wc + bc; softmax
                pz3 = p4ps.tile([8, CLS], f32, tag="pz3")
                for k in range(6):
                    pzT = p4ps.tile([128, 8], f32, tag="pzT")
                    nc.tensor.transpose(out=pzT[:], in_=z2[:, k * 128:(k + 1) * 128],
                                        identity=identity[:8, :8])
                    z2T = p4.tile([128, 8], f32, tag="z2T")
                    nc.vector.tensor_copy(z2T[:], pzT[:])
                    nc.tensor.matmul(pz3[:], lhsT=z2T[:], rhs=wc_sb[:, k, :],
                                     start=(k == 0), stop=(k == 5),
                                     skip_group_check=True)
                z3 = p4.tile([8, CLS], f32)
                nc.vector.tensor_tensor(out=z3[:], in0=pz3[:], in1=bc_sb[:], op=OP.add)
                zmax = p4.tile([8, 1], f32)
                nc.vector.tensor_reduce(out=zmax[:], in_=z3[:], axis=AX, op=OP.max)
                nc.vector.tensor_scalar(out=z3[:], in0=z3[:], scalar1=zmax[:, 0:1],
                                        scalar2=None, op0=OP.subtract)
                nc.scalar.activation(out=z3[:], in_=z3[:],
                                     func=mybir.ActivationFunctionType.Exp)
                zsum = p4.tile([8, 1], f32)
                nc.vector.tensor_reduce(out=zsum[:], in_=z3[:], axis=AX, op=OP.add)
                nc.vector.reciprocal(out=zsum[:], in_=zsum[:])
                nc.vector.tensor_scalar(out=z3[:], in0=z3[:], scalar1=zsum[:, 0:1],
                                        scalar2=None, op0=OP.mult)
                nc.sync.dma_start(out=probs_out[:], in_=z3[:])

    return nc


# ------------------------------------------------------------------- host side
def _prep(inputs):
    x = np.asarray(inputs["x"], np.float32)
    src = np.asarray(inputs["src"], np.int64)
    dst = np.asarray(inputs["dst"], np.int64)
    graph_ids = np.asarray(inputs["graph_ids"], np.int64)

    # validate the graph layout the kernel is compiled for
    gb = np.searchsorted(graph_ids, np.arange(B + 1))
    assert gb[0] == 0 and gb[B] == N
    for c in range(NCORES):
        assert gb[8 * c] == c * NLOC, "graph batches not aligned to 1250-node cores"

    # per-core edge partition, sorted by dst
    per_core = []
    Cmax = 1
    for c in range(NCORES):
        lo, hi = c * NLOC, (c + 1) * NLOC
        eids = np.nonzero((dst >= lo) & (dst < hi))[0]
        eids = eids[np.argsort(dst[eids], kind="stable")]
        dloc = (dst[eids] - lo).astype(np.int64)
        bounds = np.searchsorted(dloc, np.arange(0, NLOC + 128, 128))
        tiles = []
        for t in range(NT):
            sub = eids[bounds[t]:bounds[t + 1]]
            tiles.append(sub)
            Cmax = max(Cmax, (len(sub) + 127) // 128)
        per_core.append(tiles)
    C = Cmax

    # slot arrays: slot (t, j, p) <-> device arrays [t*128+p, j]
    src_slots = np.zeros((NCORES, NT * 128, C), np.int32)
    dl_slots = np.full((NCORES, NT * 128, C), PAD_DST, np.int32)
    eid_slots = np.full((NCORES, NT, C, 128), -1, np.int64)
    for c in range(NCORES):
        for t in range(NT):
            sub = per_core[c][t]
            n = len(sub)
            jj, pp = np.divmod(np.arange(n), 128)
            src_slots[c, t * 128 + pp, jj] = src[sub]
            dl_slots[c, t * 128 + pp, jj] = (dst[sub] - c * NLOC - t * 128)
            eid_slots[c, t, jj, pp] = sub

    # parameter prep
    W1 = np.asarray(inputs["W1"], np.float32)
    al1 = np.asarray(inputs["al1"], np.float32)
    ar1 = np.asarray(inputs["ar1"], np.float32)
    W2 = np.asarray(inputs["W2"], np.float32)
    al2 = np.asarray(inputs["al2"], np.float32)
    ar2 = np.asarray(inputs["ar2"], np.float32)
    w1aug = np.concatenate(
        [W1, np.einsum("fhd,hd->fh", W1.reshape(F, H, D), al1)], axis=1)
    war1 = np.einsum("fhd,hd->fh", W1.reshape(F, H, D), ar1)
    w2aug = np.concatenate(
        [W2, np.einsum("fhd,hd->fh", W2.reshape(D, H, D), al2)], axis=1)
    war2 = np.einsum("fhd,hd->fh", W2.reshape(D, H, D), ar2)
    b1m = np.asarray(inputs["b1"], np.float32).reshape(H, D).mean(0)[None, :]

    xT = np.ascontiguousarray(x.T)
    shared = {
        "xT": xT,
        "srcidx": None, "dstloc": None, "xT_own": None,
        "w1aug": np.ascontiguousarray(w1aug),
        "war1": np.ascontiguousarray(war1),
        "w2aug": np.ascontiguousarray(w2aug),
        "war2": np.ascontiguousarray(war2),
        "b1m": np.ascontiguousarray(b1m),
        "b2": np.asarray(inputs["b2"], np.float32)[None, :],
        "lng": np.asarray(inputs["ln_g"], np.float32)[None, :],
        "lnb": np.asarray(inputs["ln_b"], np.float32)[None, :],
        "wa": np.asarray(inputs["Wa"], np.float32),
        "ba": np.asarray(inputs["ba"], np.float32)[None, :],
        "wb": np.asarray(inputs["Wb"], np.float32),
        "bb": np.asarray(inputs["bb"], np.float32)[None, :],
        "l2g": np.asarray(inputs["ln2_g"], np.float32)[None, :],
        "l2b": np.asarray(inputs["ln2_b"], np.float32)[None, :],
        "wc": np.asarray(inputs["Wc"], np.float32),
        "bc": np.asarray(inputs["bc"], np.float32)[None, :],
    }
    in_maps = []
    for c in range(NCORES):
        m = dict(shared)
        m["srcidx"] = src_slots[c]
        m["dstloc"] = dl_slots[c]
        m["xT_own"] = np.ascontiguousarray(xT[:, c * NLOC:(c + 1) * NLOC])
        in_maps.append(m)
    return C, in_maps, eid_slots


def run(inputs, trace=False, debug=False):
    C, in_maps, eid_slots = _prep(inputs)
    nc = build_nc(C, debug=debug)
    res = run_bass_kernel_spmd(nc, in_maps, list(range(NCORES)), trace=trace)

    probs = np.concatenate([res.results[c]["probs"] for c in range(NCORES)], axis=0)
    aw = np.zeros((E, H), np.float32)
    for c in range(NCORES):
        a = res.results[c]["aw_out"].reshape(NT, 128, C, H)
        eid = eid_slots[c]                       # [NT, C, 128]
        valid = eid >= 0
        aw[eid[valid]] = a.transpose(0, 2, 1, 3)[valid]
    return (probs, aw.reshape(E, H, 1)), res


def kernel(**inputs):
    (probs, aw), _ = run(inputs)
    return probs, aw


# revision 8
# speedup vs baseline: 1.1830x; 1.1830x over previous
"""2-layer GAT + per-graph max-pool + MLP head on 8 Trainium2 cores.

Sharding: core k owns nodes [1250k, 1250k+1250) = graphs [8k, 8k+8)
(graph_ids are contiguous, 8 graphs = exactly 1250 nodes). Edges are
partitioned by dst node and sorted by dst on the host; all params and
the node-feature table are replicated so src gathers stay core-local.
Segment softmax / aggregation are computed with one-hot matmuls on the
PE array; an AllGather shares the inter-layer activations.
"""

import json

import numpy as np

import concourse.bass as bass
import concourse.mybir as mybir
import concourse.tile as tile
from concourse.bass_utils import run_bass_kernel_spmd
from concourse.masks import make_identity

# ---------------------------------------------------------------- problem dims
N, E, F, D, H, B, CLS = 10000, 160000, 128, 128, 4, 64, 2
FF = 768
SLOPE = 0.2
NCORES = 8
NLOC = N // NCORES            # 1250 nodes per core
NT = (NLOC + 127) // 128      # 10 dst tiles per core
TILE_CNT = [128] * 9 + [NLOC - 9 * 128]   # rows per dst tile
ROW = D * H + H               # 516: [h | el]
# local graph boundaries (same for every core): ceil(j * 10000/64) - 1250*k
GB = [0, 157, 313, 469, 625, 782, 938, 1094, 1250]
PAD_DST = 999                 # dst_local for padding slots (no one-hot match)

f32 = mybir.dt.float32
bf16 = mybir.dt.bfloat16
i32 = mybir.dt.int32
AX = mybir.AxisListType.X
OP = mybir.AluOpType


# ------------------------------------------------------------ BIR legalization
# This walrus build caps semaphore waits at 1 per instruction (2 for
# EventSemaphore); Tile's kernel-tail drain can carry more. Split the
# excess onto preceding single-wait NoOps on the same engine.
def _legalize_bir(bir: dict) -> None:
    caps = {"EventSemaphore": 2}
    for f in bir.get("functions", []):
        for blk in f.get("blocks", []):
            out = []
            ctr = 0
            for ins in blk.get("instructions", []):
                si = ins.get("sync_info")
                if si:
                    waits = si.get("on_wait") or []
                    cap = caps.get(ins.get("opcode"), 1)
                    if len(waits) > cap:
                        eq = [i for i, w in enumerate(waits)
                              if "eq" in (w.get("wait_mode") or "")]
                        ge = [i for i in range(len(waits)) if i not in eq]
                        assert len(eq) <= cap
                        keep = (eq + ge)[:cap]
                        for i in [i for i in ge if i not in keep]:
                            ctr += 1
                            out.append({
                                "debug": ins.get("debug"),
                                "engine": ins["engine"],
                                "ins": [], "outs": [],
                                "name": f"{ins['name']}_ws{ctr}",
                                "opcode": "NoOp",
                                "sync_info": {"on_wait": [waits[i]],
                                              "on_update": []},
                            })
                        si["on_wait"] = [waits[i] for i in sorted(keep)]
                out.append(ins)
            blk["instructions"] = out


_orig_to_json_bytes = bass.Bass.to_json_bytes


def _to_json_bytes_patched(self):
    bir = json.loads(_orig_to_json_bytes(self))
    _legalize_bir(bir)
    return json.dumps(bir).encode()


if bass.Bass.to_json_bytes is not _to_json_bytes_patched:
    bass.Bass.to_json_bytes = _to_json_bytes_patched


# ---------------------------------------------------------------- device build
def build_nc(C: int, debug: bool = False) -> bass.Bass:
    """Build the SPMD kernel; C = chunks (of 128 edge slots) per dst tile."""
    nc = bass.Bass()

    xT = nc.dram_tensor("xT", [F, N], f32, kind="ExternalInput")
    xT_own = nc.dram_tensor("xT_own", [F, NLOC], f32, kind="ExternalInput")
    srcidx = nc.dram_tensor("srcidx", [NT * 128, C], i32, kind="ExternalInput")
    dstloc = nc.dram_tensor("dstloc", [NT * 128, C], i32, kind="ExternalInput")
    w1aug = nc.dram_tensor("w1aug", [F, ROW], f32, kind="ExternalInput")
    war1 = nc.dram_tensor("war1", [F, H], f32, kind="ExternalInput")
    w2aug = nc.dram_tensor("w2aug", [D, ROW], f32, kind="ExternalInput")
    war2 = nc.dram_tensor("war2", [D, H], f32, kind="ExternalInput")
    b1m_d = nc.dram_tensor("b1m", [1, D], f32, kind="ExternalInput")
    b2_d = nc.dram_tensor("b2", [1, H * D], f32, kind="ExternalInput")
    lng_d = nc.dram_tensor("lng", [1, D], f32, kind="ExternalInput")
    lnb_d = nc.dram_tensor("lnb", [1, D], f32, kind="ExternalInput")
    wa_d = nc.dram_tensor("wa", [H * D, D], f32, kind="ExternalInput")
    ba_d = nc.dram_tensor("ba", [1, D], f32, kind="ExternalInput")
    wb_d = nc.dram_tensor("wb", [D, FF], f32, kind="ExternalInput")
    bb_d = nc.dram_tensor("bb", [1, FF], f32, kind="ExternalInput")
    l2g_d = nc.dram_tensor("l2g", [1, FF], f32, kind="ExternalInput")
    l2b_d = nc.dram_tensor("l2b", [1, FF], f32, kind="ExternalInput")
    wc_d = nc.dram_tensor("wc", [FF, CLS], f32, kind="ExternalInput")
    bc_d = nc.dram_tensor("bc", [1, CLS], f32, kind="ExternalInput")

    aw_out = nc.dram_tensor("aw_out", [NT * 128, C * H], f32, kind="ExternalOutput")
    if debug:
        dbg_haug1 = nc.dram_tensor("dbg_haug1", [N, ROW], f32, kind="ExternalOutput")
        dbg_x2 = nc.dram_tensor("dbg_x2", [NLOC, D], f32, kind="ExternalOutput")
        dbg_x2full = nc.dram_tensor("dbg_x2full", [N, D], f32, kind="ExternalOutput")
        dbg_rden1 = nc.dram_tensor("dbg_rden1", [NT * 128, H], f32, kind="ExternalOutput")
        dbg_h2 = nc.dram_tensor("dbg_h2", [NT * 128, 512], f32, kind="ExternalOutput")
        dbg_er2 = nc.dram_tensor("dbg_er2", [NT * 128, H], f32, kind="ExternalOutput")
        dbg_agg = nc.dram_tensor("dbg_agg", [NT * 128, 512], f32, kind="ExternalOutput")
    probs_out = nc.dram_tensor("probs", [B // NCORES, CLS], f32, kind="ExternalOutput")

    def bcast(dram_t, parts, k):
        # [1, k] DRAM row -> [parts, k] via partition-stride-0 DMA
        return bass.AP(tensor=dram_t[:].tensor, offset=0, ap=[[0, parts], [1, k]])

    with tile.TileContext(nc) as tc:
        with (
            tc.tile_pool(name="consts", bufs=1) as consts,
            tc.tile_pool(name="dram", bufs=1, space="DRAM") as dram,
            tc.tile_pool(name="persist", bufs=1) as persist,
        ):
            # ---------------- constants
            identity = consts.tile([128, 128], f32)
            make_identity(nc, identity[:])
            iota_row_i = consts.tile([128, 128], i32)
            nc.gpsimd.iota(iota_row_i[:], pattern=[[1, 128]], channel_multiplier=0)
            iota_row = consts.tile([128, 128], f32)
            nc.vector.tensor_copy(iota_row[:], iota_row_i[:])
            iota_col_i = consts.tile([128, 1], i32)
            nc.gpsimd.iota(iota_col_i[:], pattern=[[0, 1]], channel_multiplier=1)
            iota_col = consts.tile([128, 1], f32)
            nc.vector.tensor_copy(iota_col[:], iota_col_i[:])
            ones_col = consts.tile([128, 1], f32)
            nc.vector.memset(ones_col[:], 1.0)

            w1aug_sb = consts.tile([F, ROW], f32)
            nc.sync.dma_start(out=w1aug_sb[:], in_=w1aug[:])
            war1_sb = consts.tile([F, H], f32)
            nc.sync.dma_start(out=war1_sb[:], in_=war1[:])
            w2aug_sb = consts.tile([D, ROW], f32)
            nc.sync.dma_start(out=w2aug_sb[:], in_=w2aug[:])
            war2_sb = consts.tile([D, H], f32)
            nc.sync.dma_start(out=war2_sb[:], in_=war2[:])
            b1m_sb = consts.tile([128, D], f32)
            nc.sync.dma_start(out=b1m_sb[:], in_=bcast(b1m_d, 128, D))
            b2_sb = consts.tile([128, H * D], f32)
            nc.sync.dma_start(out=b2_sb[:], in_=bcast(b2_d, 128, H * D))
            lng_sb = consts.tile([128, D], f32)
            nc.sync.dma_start(out=lng_sb[:], in_=bcast(lng_d, 128, D))
            lnb_sb = consts.tile([128, D], f32)
            nc.sync.dma_start(out=lnb_sb[:], in_=bcast(lnb_d, 128, D))
            wa_sb = consts.tile([128, H, D], f32)
            nc.sync.dma_start(out=wa_sb[:], in_=wa_d.rearrange("(b k) n -> k b n", k=128))
            ba_sb = consts.tile([8, D], f32)
            nc.sync.dma_start(out=ba_sb[:], in_=bcast(ba_d, 8, D))
            wb_sb = consts.tile([D, FF], f32)
            nc.sync.dma_start(out=wb_sb[:], in_=wb_d[:])
            bb_sb = consts.tile([8, FF], f32)
            nc.sync.dma_start(out=bb_sb[:], in_=bcast(bb_d, 8, FF))
            l2g_sb = consts.tile([8, FF], f32)
            nc.sync.dma_start(out=l2g_sb[:], in_=bcast(l2g_d, 8, FF))
            l2b_sb = consts.tile([8, FF], f32)
            nc.sync.dma_start(out=l2b_sb[:], in_=bcast(l2b_d, 8, FF))
            wc_sb = consts.tile([128, 6, CLS], f32)
            nc.sync.dma_start(out=wc_sb[:], in_=wc_d.rearrange("(b k) n -> k b n", k=128))
            bc_sb = consts.tile([8, CLS], f32)
            nc.sync.dma_start(out=bc_sb[:], in_=bcast(bc_d, 8, CLS))
            xT_own_sb = consts.tile([F, NLOC], f32)
            nc.sync.dma_start(out=xT_own_sb[:], in_=xT_own[:])

            # persistent state
            er2_all = persist.tile([128, NT, H], f32)
            pooledT = persist.tile([128, H, 8], f32)
            nc.vector.memset(pooledT[:], -3.0e38)

            # internal DRAM
            haug1 = dram.tile([N, ROW], f32)
            haug2 = dram.tile([N, ROW], f32)
            x2_shard = dram.tile([NLOC, D], f32)
            x2_full = dram.tile([N, D], f32, addr_space="Shared")

            # ============================ P0: haug1 = x @ [W1|Wal1], full N
            with (
                tc.tile_pool(name="p0", bufs=2) as p0,
                tc.tile_pool(name="p0ps", bufs=2, space="PSUM") as p0ps,
                tc.tile_pool(name="xtp", bufs=1) as xtp,
            ):
                xT_sb = xtp.tile([F, N], f32)
                nc.sync.dma_start(out=xT_sb[:], in_=xT[:])
                for i in range((N + 127) // 128):
                    cnt = min(128, N - i * 128)
                    ph = p0ps.tile([128, 512], f32, tag="ph")
                    pe = p0ps.tile([128, H], f32, tag="pe")
                    lhs = xT_sb[:, i * 128:i * 128 + cnt]
                    nc.tensor.matmul(ph[:cnt, :], lhsT=lhs, rhs=w1aug_sb[:, 0:512],
                                     start=True, stop=True)
                    nc.tensor.matmul(pe[:cnt, :], lhsT=lhs, rhs=w1aug_sb[:, 512:ROW],
                                     start=True, stop=True)
                    hrow = p0.tile([128, ROW], f32, tag="hrow")
                    nc.scalar.copy(out=hrow[:cnt, 0:512], in_=ph[:cnt, :])
                    nc.vector.tensor_copy(out=hrow[:cnt, 512:ROW], in_=pe[:cnt, :])
                    nc.sync.dma_start(out=haug1[i * 128:i * 128 + cnt, :],
                                      in_=hrow[:cnt, :])

            # ============================ edge-loop helper (shared by layers)
            def edge_layer(haug, war_sb, layer):
                """Per dst tile: gather src rows, attention, aggregate.

                layer=1: returns per-tile x2 (LN'd head-mean) written to
                x2_shard + er2_all. layer=2: writes aw, updates pooledT.
                """
                with (
                    tc.tile_pool(name=f"hs{layer}", bufs=2) as hsp,
                    tc.tile_pool(name=f"oh{layer}", bufs=2) as ohp,
                    tc.tile_pool(name=f"wk{layer}", bufs=3) as wk,
                    tc.tile_pool(name=f"soh{layer}", bufs=4) as sohp,
                    tc.tile_pool(name=f"sm{layer}", bufs=8) as sm,
                    tc.tile_pool(name=f"ps{layer}", bufs=2, space="PSUM") as ps,
                    tc.tile_pool(name=f"pt{layer}", bufs=2, space="PSUM") as pst,
                ):
                    for t in range(NT):
                        cnt = TILE_CNT[t]
                        r0 = t * 128
                        # --- load index data
                        idx_sb = wk.tile([128, C], i32, tag="idx")
                        nc.sync.dma_start(out=idx_sb[:], in_=srcidx[r0:r0 + 128, :])
                        dl_sb = wk.tile([128, C], i32, tag="dl")
                        nc.sync.dma_start(out=dl_sb[:], in_=dstloc[r0:r0 + 128, :])
                        dlf = wk.tile([128, C], f32, tag="dlf")
                        nc.vector.tensor_copy(dlf[:], dl_sb[:])
                        # --- er for this tile's dst rows
                        er_sb = sm.tile([128, H], f32, tag="er")
                        nc.vector.memset(er_sb[:], 0.0)
                        per = pst.tile([128, H], f32, tag="per", bufs=1)
                        if layer == 1:
                            nc.tensor.matmul(per[:cnt, :],
                                             lhsT=xT_own_sb[:, r0:r0 + cnt],
                                             rhs=war_sb[:], start=True, stop=True)
                            nc.vector.tensor_copy(er_sb[:cnt, :], per[:cnt, :])
                        else:
                            nc.vector.tensor_copy(er_sb[:cnt, :], er2_all[:cnt, t, :])
                        # --- gather src rows (h | el)
                        hsrc = hsp.tile([128, C, ROW], f32, tag="hsrc")
                        for j in range(C):
                            nc.gpsimd.indirect_dma_start(
                                out=hsrc[:, j, :], out_offset=None, in_=haug[:],
                                in_offset=bass.IndirectOffsetOnAxis(
                                    ap=idx_sb[:, j:j + 1], axis=0),
                            )
                        ohT = ohp.tile([128, C * 128], f32, tag="ohT")
                        extile = wk.tile([128, C * H], f32, tag="ex")
                        pagg = ps.tile([128, 512], f32, tag="pagg", bufs=1)
                        pden = ps.tile([128, H], f32, tag="pden", bufs=1)
                        for j in range(C):
                            # transposed dst_local -> one-hot^T [d, e]
                            pdT = pst.tile([128, 128], f32, tag="pdT")
                            nc.tensor.transpose(
                                out=pdT[:], in_=dlf[:, j:j + 1].to_broadcast([128, 128]),
                                identity=identity[:])
                            ohTj = ohT[:, j * 128:(j + 1) * 128]
                            nc.vector.tensor_tensor(
                                out=ohTj, in0=pdT[:],
                                in1=iota_col[:, 0:1].to_broadcast([128, 128]),
                                op=OP.is_equal)
                            # er broadcast to edges: ohT^T @ er
                            perb = pst.tile([128, H], f32, tag="perb")
                            nc.tensor.matmul(perb[:], lhsT=ohTj, rhs=er_sb[:],
                                             start=True, stop=True)
                            # e = leaky(el_src + er_dst); ex = exp(e)
                            ej = sm.tile([128, H], f32, tag="ej")
                            nc.vector.tensor_tensor(out=ej[:], in0=hsrc[:, j, 512:ROW],
                                                    in1=perb[:], op=OP.add)
                            nc.vector.scalar_tensor_tensor(
                                out=ej[:], in0=ej[:], scalar=SLOPE, in1=ej[:],
                                op0=OP.mult, op1=OP.max)
                            exj = extile[:, j * H:(j + 1) * H]
                            nc.scalar.activation(out=exj, in_=ej[:],
                                                 func=mybir.ActivationFunctionType.Exp)
                            # one-hot + scaled rhs aggregation matmuls
                            oh = sohp.tile([128, 128], bf16, tag="oh")
                            nc.vector.tensor_scalar(
                                out=oh[:], in0=iota_row[:],
                                scalar1=dlf[:, j:j + 1], scalar2=None,
                                op0=OP.is_equal)
                            exjb = sm.tile([128, H], bf16, tag="exjb")
                            nc.vector.tensor_copy(exjb[:], exj)
                            nc.tensor.matmul(
                                pden[:], lhsT=oh[:], rhs=exjb[:],
                                start=(j == 0), stop=(j == C - 1),
                                skip_group_check=True)
                            sh = sohp.tile([128, 512], bf16, tag="sh")
                            for h in range(H):
                                if h % 2 == 0:
                                    nc.scalar.activation(
                                        out=sh[:, h * 128:(h + 1) * 128],
                                        in_=hsrc[:, j, h * 128:(h + 1) * 128],
                                        func=mybir.ActivationFunctionType.Copy,
                                        scale=extile[:, j * H + h:j * H + h + 1])
                                else:
                                    nc.vector.tensor_scalar(
                                        out=sh[:, h * 128:(h + 1) * 128],
                                        in0=hsrc[:, j, h * 128:(h + 1) * 128],
                                        scalar1=extile[:, j * H + h:j * H + h + 1],
                                        scalar2=None, op0=OP.mult)
                            nc.tensor.matmul(
                                pagg[:], lhsT=oh[:], rhs=sh[:],
                                start=(j == 0), stop=(j == C - 1),
                                skip_group_check=True)
                        # --- denominators
                        rden = sm.tile([128, H], f32, tag="rden")
                        nc.vector.tensor_scalar(out=rden[:], in0=pden[:], scalar1=1e-9,
                                                scalar2=None, op0=OP.max)
                        nc.vector.reciprocal(out=rden[:], in_=rden[:])
                        if debug and layer == 1:
                            nc.sync.dma_start(out=dbg_rden1[r0:r0 + 128, :], in_=rden[:])
                            aggd = wk.tile([128, 512], f32, tag="aggd")
                            nc.vector.tensor_copy(aggd[:], pagg[:])
                            nc.sync.dma_start(out=dbg_agg[r0:r0 + 128, :], in_=aggd[:])

                        if layer == 1:
                            # hm = 0.25 * sum_h agg_h*rden_h + b1m
                            m0 = wk.tile([128, D], f32, tag="m0")
                            m1 = wk.tile([128, D], f32, tag="m1")
                            nc.scalar.activation(out=m0[:], in_=pagg[:, 0:128],
                                                 func=mybir.ActivationFunctionType.Copy,
                                                 scale=rden[:, 0:1])
                            nc.scalar.activation(out=m1[:], in_=pagg[:, 128:256],
                                                 func=mybir.ActivationFunctionType.Copy,
                                                 scale=rden[:, 1:2])
                            nc.vector.tensor_add(m0[:], m0[:], m1[:])
                            nc.scalar.activation(out=m1[:], in_=pagg[:, 256:384],
                                                 func=mybir.ActivationFunctionType.Copy,
                                                 scale=rden[:, 2:3])
                            nc.vector.tensor_add(m0[:], m0[:], m1[:])
                            nc.scalar.activation(out=m1[:], in_=pagg[:, 384:512],
                                                 func=mybir.ActivationFunctionType.Copy,
                                                 scale=rden[:, 3:4])
                            nc.vector.tensor_add(m0[:], m0[:], m1[:])
                            hm = wk.tile([128, D], f32, tag="hm")
                            nc.vector.scalar_tensor_tensor(
                                out=hm[:], in0=m0[:], scalar=0.25, in1=b1m_sb[:],
                                op0=OP.mult, op1=OP.add)
                            # LayerNorm over D
                            mu = sm.tile([128, 1], f32, tag="mu")
                            nc.vector.tensor_reduce(out=mu[:], in_=hm[:], axis=AX,
                                                    op=OP.add)
                            nc.scalar.mul(out=mu[:], in_=mu[:], mul=1.0 / D)
                            cen = wk.tile([128, D], f32, tag="cen")
                            nc.vector.tensor_scalar(out=cen[:], in0=hm[:],
                                                    scalar1=mu[:, 0:1], scalar2=None,
                                                    op0=OP.subtract)
                            sq = wk.tile([128, D], f32, tag="sq")
                            nc.vector.tensor_tensor(out=sq[:], in0=cen[:], in1=cen[:],
                                                    op=OP.mult)
                            var = sm.tile([128, 1], f32, tag="var")
                            nc.vector.tensor_reduce(out=var[:], in_=sq[:], axis=AX,
                                                    op=OP.add)
                            std = sm.tile([128, 1], f32, tag="std")
                            nc.vector.tensor_scalar(out=var[:], in0=var[:],
                                                    scalar1=1.0 / D, scalar2=1e-5,
                                                    op0=OP.mult, op1=OP.add)
                            nc.scalar.activation(out=std[:], in_=var[:],
                                                 func=mybir.ActivationFunctionType.Sqrt)
                            nc.vector.reciprocal(out=std[:], in_=std[:])
                            x2t = wk.tile([128, D], f32, tag="x2t")
                            nc.vector.tensor_scalar(out=x2t[:], in0=cen[:],
                                                    scalar1=std[:, 0:1], scalar2=None,
                                                    op0=OP.mult)
                            nc.vector.tensor_tensor(out=x2t[:], in0=x2t[:],
                                                    in1=lng_sb[:], op=OP.mult)
                            nc.vector.tensor_tensor(out=x2t[:], in0=x2t[:],
                                                    in1=lnb_sb[:], op=OP.add)
                            nc.vector.scalar_tensor_tensor(
                                out=x2t[:], in0=x2t[:], scalar=SLOPE, in1=x2t[:],
                                op0=OP.mult, op1=OP.max)
                            nc.sync.dma_start(out=x2_shard[r0:r0 + cnt, :],
                                              in_=x2t[:cnt, :])
                            if debug:
                                nc.sync.dma_start(out=dbg_x2[r0:r0 + cnt, :],
                                                  in_=x2t[:cnt, :])
                            # er2 for this tile = x2 @ War2 (needs x2^T)
                            pxT = pst.tile([128, 128], f32, tag="pdT")
                            nc.tensor.transpose(out=pxT[:, :cnt], in_=x2t[:cnt, :],
                                                identity=identity[:cnt, :cnt])
                            x2T = wk.tile([128, 128], f32, tag="x2T")
                            nc.vector.memset(x2T[:], 0.0)
                            nc.vector.tensor_copy(x2T[:, :cnt], pxT[:, :cnt])
                            per2 = pst.tile([128, H], f32, tag="per", bufs=1)
                            nc.tensor.matmul(per2[:cnt, :], lhsT=x2T[:, :cnt],
                                             rhs=war2_sb[:], start=True, stop=True)
                            nc.vector.memset(er2_all[:, t, :], 0.0)
                            nc.vector.tensor_copy(er2_all[:cnt, t, :], per2[:cnt, :])
                            if debug:
                                nc.sync.dma_start(out=dbg_er2[r0:r0 + 128, :],
                                                  in_=er2_all[:, t, :])
                        else:
                            # h2 = leaky(agg*rden + b2)
                            h2 = wk.tile([128, 512], f32, tag="h2")
                            for h in range(H):
                                blk = h2[:, h * 128:(h + 1) * 128]
                                nc.scalar.activation(
                                    out=blk, in_=pagg[:, h * 128:(h + 1) * 128],
                                    func=mybir.ActivationFunctionType.Copy,
                                    scale=rden[:, h:h + 1])
                            nc.vector.tensor_tensor(out=h2[:], in0=h2[:], in1=b2_sb[:],
                                                    op=OP.add)
                            nc.vector.scalar_tensor_tensor(
                                out=h2[:], in0=h2[:], scalar=SLOPE, in1=h2[:],
                                op0=OP.mult, op1=OP.max)
                            if debug:
                                nc.sync.dma_start(out=dbg_h2[r0:r0 + 128, :], in_=h2[:])
                            # attention weights out: a = ex * rden[dst]
                            aw_sb = wk.tile([128, C * H], f32, tag="aw")
                            for j in range(C):
                                pa = pst.tile([128, H], f32, tag="perb")
                                nc.tensor.matmul(pa[:],
                                                 lhsT=ohT[:, j * 128:(j + 1) * 128],
                                                 rhs=rden[:], start=True, stop=True)
                                nc.vector.tensor_tensor(
                                    out=aw_sb[:, j * H:(j + 1) * H],
                                    in0=extile[:, j * H:(j + 1) * H], in1=pa[:],
                                    op=OP.mult)
                            nc.sync.dma_start(out=aw_out[r0:r0 + 128, :], in_=aw_sb[:])
                            # per-graph max pool (feature-major via PE transpose)
                            for h in range(H):
                                pT = pst.tile([128, 128], f32, tag="pdT")
                                nc.tensor.transpose(
                                    out=pT[:, :cnt], in_=h2[:cnt, h * 128:(h + 1) * 128],
                                    identity=identity[:cnt, :cnt])
                                for g in range(8):
                                    lo = max(GB[g], r0) - r0
                                    hi = min(GB[g + 1], r0 + cnt) - r0
                                    if lo >= hi:
                                        continue
                                    red = sm.tile([128, 1], f32, tag="red")
                                    nc.vector.tensor_reduce(
                                        out=red[:], in_=pT[:, lo:hi], axis=AX, op=OP.max)
                                    nc.vector.tensor_tensor(
                                        out=pooledT[:, h, g:g + 1],
                                        in0=pooledT[:, h, g:g + 1], in1=red[:],
                                        op=OP.max)

            # ============================ P1: layer-1 edge loop
            edge_layer(haug1, war1_sb, layer=1)

            # ============================ AllGather x2 shards
            nc.gpsimd.collective_compute(
                "AllGather", OP.bypass,
                replica_groups=[list(range(NCORES))],
                ins=[x2_shard.opt()], outs=[x2_full.opt()],
            )

            if debug:
                with tc.tile_pool(name="dbgp", bufs=2) as dbgp:
                    for i in range((N + 127) // 128):
                        cnt = min(128, N - i * 128)
                        tmp = dbgp.tile([128, ROW], f32, tag="tmp")
                        nc.sync.dma_start(out=tmp[:cnt, :], in_=haug1[i * 128:i * 128 + cnt, :])
                        nc.sync.dma_start(out=dbg_haug1[i * 128:i * 128 + cnt, :], in_=tmp[:cnt, :])
                        tmp2 = dbgp.tile([128, D], f32, tag="tmp2")
                        nc.sync.dma_start(out=tmp2[:cnt, :], in_=x2_full[i * 128:i * 128 + cnt, :])
                        nc.sync.dma_start(out=dbg_x2full[i * 128:i * 128 + cnt, :], in_=tmp2[:cnt, :])

            # ============================ P2: haug2 = x2 @ [W2|Wal2], full N
            with (
                tc.tile_pool(name="p2", bufs=3) as p2,
                tc.tile_pool(name="p2ps", bufs=2, space="PSUM") as p2ps,
            ):
                for s in range(NCORES):
                    for t in range(NT):
                        cnt = TILE_CNT[t]
                        g0 = s * NLOC + t * 128
                        x2t_sb = p2.tile([128, D], f32, tag="x2t")
                        nc.sync.dma_start(out=x2t_sb[:cnt, :],
                                          in_=x2_full[g0:g0 + cnt, :])
                        pT = p2ps.tile([128, 128], f32, tag="pT")
                        nc.tensor.transpose(out=pT[:, :cnt], in_=x2t_sb[:cnt, :],
                                            identity=identity[:cnt, :cnt])
                        x2T_sb = p2.tile([128, 128], f32, tag="x2T")
                        nc.scalar.copy(out=x2T_sb[:, :cnt], in_=pT[:, :cnt])
                        ph = p2ps.tile([128, 512], f32, tag="ph")
                        pe = p2ps.tile([128, H], f32, tag="pe")
                        nc.tensor.matmul(ph[:cnt, :], lhsT=x2T_sb[:, :cnt],
                                         rhs=w2aug_sb[:, 0:512], start=True, stop=True)
                        nc.tensor.matmul(pe[:cnt, :], lhsT=x2T_sb[:, :cnt],
                                         rhs=w2aug_sb[:, 512:ROW], start=True, stop=True)
                        hrow = p2.tile([128, ROW], f32, tag="hrow")
                        nc.scalar.copy(out=hrow[:cnt, 0:512], in_=ph[:cnt, :])
                        nc.vector.tensor_copy(out=hrow[:cnt, 512:ROW], in_=pe[:cnt, :])
                        nc.sync.dma_start(out=haug2[g0:g0 + cnt, :], in_=hrow[:cnt, :])

            # ============================ P3: layer-2 edge loop
            edge_layer(haug2, war2_sb, layer=2)

            # ============================ P4: head MLP on pooled [8, 512]
            with (
                tc.tile_pool(name="p4", bufs=1) as p4,
                tc.tile_pool(name="p4ps", bufs=1, space="PSUM") as p4ps,
            ):
                pz1 = p4ps.tile([8, D], f32, tag="pz1")
                for h in range(H):
                    nc.tensor.matmul(pz1[:], lhsT=pooledT[:, h, :], rhs=wa_sb[:, h, :],
                                     start=(h == 0), stop=(h == H - 1))
                z1 = p4.tile([8, D], f32)
                nc.vector.tensor_tensor(out=z1[:], in0=pz1[:], in1=ba_sb[:], op=OP.add)
                pz1T = p4ps.tile([128, 8], f32, tag="pz1T")
                nc.tensor.transpose(out=pz1T[:], in_=z1[:], identity=identity[:8, :8])
                z1T = p4.tile([128, 8], f32)
                nc.vector.tensor_copy(z1T[:], pz1T[:])
                pz2a = p4ps.tile([8, 384], f32, tag="pz2a")
                pz2b = p4ps.tile([8, 384], f32, tag="pz2b")
                nc.tensor.matmul(pz2a[:], lhsT=z1T[:], rhs=wb_sb[:, 0:384],
                                 start=True, stop=True)
                nc.tensor.matmul(pz2b[:], lhsT=z1T[:], rhs=wb_sb[:, 384:FF],
                                 start=True, stop=True)
                z2 = p4.tile([8, FF], f32)
                nc.vector.tensor_copy(z2[:, 0:384], pz2a[:])
                nc.vector.tensor_copy(z2[:, 384:FF], pz2b[:])
                nc.vector.tensor_tensor(out=z2[:], in0=z2[:], in1=bb_sb[:], op=OP.add)
                # LayerNorm over FF
                mu = p4.tile([8, 1], f32)
                nc.vector.tensor_reduce(out=mu[:], in_=z2[:], axis=AX, op=OP.add)
                nc.scalar.mul(out=mu[:], in_=mu[:], mul=1.0 / FF)
                nc.vector.tensor_scalar(out=z2[:], in0=z2[:], scalar1=mu[:, 0:1],
                                        scalar2=None, op0=OP.subtract)
                sq = p4.tile([8, FF], f32)
                nc.vector.tensor_tensor(out=sq[:], in0=z2[:], in1=z2[:], op=OP.mult)
                var = p4.tile([8, 1], f32)
                nc.vector.tensor_reduce(out=var[:], in_=sq[:], axis=AX, op=OP.add)
                std = p4.tile([8, 1], f32)
                nc.vector.tensor_scalar(out=var[:], in0=var[:],
                                        scalar1=1.0 / FF, scalar2=1e-5,
                                        op0=OP.mult, op1=OP.add)
                nc.scalar.activation(out=std[:], in_=var[:],
                                     func=mybir.ActivationFunctionType.Sqrt)
                nc.vector.reciprocal(out=std[:], in_=std[:])
                nc.vector.tensor_scalar(out=z2[:], in0=z2[:], scalar1=std[:, 0:1],
                                        scalar2=None, op0=OP.mult)
                nc.vector.tensor_tensor(out=z2[:], in0=z2[:], in1=l2g_sb[:], op=OP.mult)
                nc.vector.tensor_tensor(out=z2[:], in0=z2[:], in1=l2b_sb[:], op=OP.add)
                nc.vector.scalar_tensor_tensor(out=z2[:], in0=z2[:], scalar=SLOPE,
                                               in1=z2[:], op0=OP.mult, op1=OP.max)
                # z3 = z2 @ Wc + bc; softmax
                pz3 = p4ps.tile([8, CLS], f32, tag="pz3")
                for k in range(6):
                    pzT = p4ps.tile([128, 8], f32, tag="pzT")
                    nc.tensor.transpose(out=pzT[:], in_=z2[:, k * 128:(k + 1) * 128],
                                        identity=identity[:8, :8])
                    z2T = p4.tile([128, 8], f32, tag="z2T")
                    nc.vector.tensor_copy(z2T[:], pzT[:])
                    nc.tensor.matmul(pz3[:], lhsT=z2T[:], rhs=wc_sb[:, k, :],
                                     start=(k == 0), stop=(k == 5),
                                     skip_group_check=True)
                z3 = p4.tile([8, CLS], f32)
                nc.vector.tensor_tensor(out=z3[:], in0=pz3[:], in1=bc_sb[:], op=OP.add)
                zmax = p4.tile([8, 1], f32)
                nc.vector.tensor_reduce(out=zmax[:], in_=z3[:], axis=AX, op=OP.max)
                nc.vector.tensor_scalar(out=z3[:], in0=z3[:], scalar1=zmax[:, 0:1],
                                        scalar2=None, op0=OP.subtract)
                nc.scalar.activation(out=z3[:], in_=z3[:],
                                     func=mybir.ActivationFunctionType.Exp)
                zsum = p4.tile([8, 1], f32)
                nc.vector.tensor_reduce(out=zsum[:], in_=z3[:], axis=AX, op=OP.add)
                nc.vector.reciprocal(out=zsum[:], in_=zsum[:])
                nc.vector.tensor_scalar(out=z3[:], in0=z3[:], scalar1=zsum[:, 0:1],
                                        scalar2=None, op0=OP.mult)
                nc.sync.dma_start(out=probs_out[:], in_=z3[:])

    return nc


# ------------------------------------------------------------------- host side
def _prep(inputs):
    x = np.asarray(inputs["x"], np.float32)
    src = np.asarray(inputs["src"], np.int64)
    dst = np.asarray(inputs["dst"], np.int64)
    graph_ids = np.asarray(inputs["graph_ids"], np.int64)

    # validate the graph layout the kernel is compiled for
    gb = np.searchsorted(graph_ids, np.arange(B + 1))
    assert gb[0] == 0 and gb[B] == N
    for c in range(NCORES):
        assert gb[8 * c] == c * NLOC, "graph batches not aligned to 1250-node cores"

    # per-core edge partition, sorted by dst
    per_core = []
    Cmax = 1
    for c in range(NCORES):
        lo, hi = c * NLOC, (c + 1) * NLOC
        eids = np.nonzero((dst >= lo) & (dst < hi))[0]
        eids = eids[np.argsort(dst[eids], kind="stable")]
        dloc = (dst[eids] - lo).astype(np.int64)
        bounds = np.searchsorted(dloc, np.arange(0, NLOC + 128, 128))
        tiles = []
        for t in range(NT):
            sub = eids[bounds[t]:bounds[t + 1]]
            tiles.append(sub)
            Cmax = max(Cmax, (len(sub) + 127) // 128)
        per_core.append(tiles)
    C = Cmax

    # slot arrays: slot (t, j, p) <-> device arrays [t*128+p, j]
    src_slots = np.zeros((NCORES, NT * 128, C), np.int32)
    dl_slots = np.full((NCORES, NT * 128, C), PAD_DST, np.int32)
    eid_slots = np.full((NCORES, NT, C, 128), -1, np.int64)
    for c in range(NCORES):
        for t in range(NT):
            sub = per_core[c][t]
            n = len(sub)
            jj, pp = np.divmod(np.arange(n), 128)
            src_slots[c, t * 128 + pp, jj] = src[sub]
            dl_slots[c, t * 128 + pp, jj] = (dst[sub] - c * NLOC - t * 128)
            eid_slots[c, t, jj, pp] = sub

    # parameter prep
    W1 = np.asarray(inputs["W1"], np.float32)
    al1 = np.asarray(inputs["al1"], np.float32)
    ar1 = np.asarray(inputs["ar1"], np.float32)
    W2 = np.asarray(inputs["W2"], np.float32)
    al2 = np.asarray(inputs["al2"], np.float32)
    ar2 = np.asarray(inputs["ar2"], np.float32)
    w1aug = np.concatenate(
        [W1, np.einsum("fhd,hd->fh", W1.reshape(F, H, D), al1)], axis=1)
    war1 = np.einsum("fhd,hd->fh", W1.reshape(F, H, D), ar1)
    w2aug = np.concatenate(
        [W2, np.einsum("fhd,hd->fh", W2.reshape(D, H, D), al2)], axis=1)
    war2 = np.einsum("fhd,hd->fh", W2.reshape(D, H, D), ar2)
    b1m = np.asarray(inputs["b1"], np.float32).reshape(H, D).mean(0)[None, :]

    xT = np.ascontiguousarray(x.T)
    shared = {
        "xT": xT,
        "srcidx": None, "dstloc": None, "xT_own": None,
        "w1aug": np.ascontiguousarray(w1aug),
        "war1": np.ascontiguousarray(war1),
        "w2aug": np.ascontiguousarray(w2aug),
        "war2": np.ascontiguousarray(war2),
        "b1m": np.ascontiguousarray(b1m),
        "b2": np.asarray(inputs["b2"], np.float32)[None, :],
        "lng": np.asarray(inputs["ln_g"], np.float32)[None, :],
        "lnb": np.asarray(inputs["ln_b"], np.float32)[None, :],
        "wa": np.asarray(inputs["Wa"], np.float32),
        "ba": np.asarray(inputs["ba"], np.float32)[None, :],
        "wb": np.asarray(inputs["Wb"], np.float32),
        "bb": np.asarray(inputs["bb"], np.float32)[None, :],
        "l2g": np.asarray(inputs["ln2_g"], np.float32)[None, :],
        "l2b": np.asarray(inputs["ln2_b"], np.float32)[None, :],
        "wc": np.asarray(inputs["Wc"], np.float32),
        "bc": np.asarray(inputs["bc"], np.float32)[None, :],
    }
    in_maps = []
    for c in range(NCORES):
        m = dict(shared)
        m["srcidx"] = src_slots[c]
        m["dstloc"] = dl_slots[c]
        m["xT_own"] = np.ascontiguousarray(xT[:, c * NLOC:(c + 1) * NLOC])
        in_maps.append(m)
    return C, in_maps, eid_slots


def run(inputs, trace=False, debug=False):
    C, in_maps, eid_slots = _prep(inputs)
    nc = build_nc(C, debug=debug)
    res = run_bass_kernel_spmd(nc, in_maps, list(range(NCORES)), trace=trace)

    probs = np.concatenate([res.results[c]["probs"] for c in range(NCORES)], axis=0)
    aw = np.zeros((E, H), np.float32)
    for c in range(NCORES):
        a = res.results[c]["aw_out"].reshape(NT, 128, C, H)
        eid = eid_slots[c]                       # [NT, C, 128]
        valid = eid >= 0
        aw[eid[valid]] = a.transpose(0, 2, 1, 3)[valid]
    return (probs, aw.reshape(E, H, 1)), res


def kernel(**inputs):
    (probs, aw), _ = run(inputs)
    return probs, aw
